# revision 9
# baseline (speedup 1.0000x reference)
"""AttentionalPooler Trainium2 kernel.

Full inputs -> full output; batch (8) is data-parallel across the 8
NeuronCores. Per core: LayerNorm(x_b), kv = LN(x_b) @ Wkv, 12-head
cross-attention from 256 pre-computed queries, output projection.

Host-side preprocessing (exact fp32 algebra, batch-independent):
  - q path (LN(query) @ Wq * dh^-0.5, transposed) is computed on host.
  - ln_k_w is folded into the kv weights (Wp = diag(ln_k_w) @ Wkv).
  - ln_k_b folds into c = ln_k_b @ Wkv. The k-part of c shifts every
    logit of a (head, query) row by the same constant, which softmax
    cancels exactly, so it is dropped. The v-part adds c_v to every
    attention output row (attention weights sum to 1), so it commutes
    past Wout: the kernel adds r = c_v @ Wout to the final output.

Device layout choices:
  - sim is computed transposed (simT[tok, query]) so both attention
    matmuls take operands in their natural layouts; softmax max-
    subtraction is skipped (logits are provably tiny for LN'd inputs),
    and denominators come from a ones-column appended to v.
  - x tiles are normalized on DVE, cast to bf16, and transposed via
    DMA-transpose into xnT; all matmuls run in bf16 with fp32 psum.
"""

import sys

sys.path.insert(0, "/opt/trn_rl_repo")

import numpy as np
import ml_dtypes

import concourse.bass as bass
import concourse.mybir as mybir
import concourse.tile as tile
from concourse import bacc
from concourse.bass_utils import run_bass_kernel_spmd

F32 = mybir.dt.float32
BF16 = mybir.dt.bfloat16
AX = mybir.AluOpType

B = 8
N_TOK = 4096
D_CTX = 1024
D_MODEL = 768
N_HEAD = 12
DH = 64
NQ = 256
INNER = 768
EPS = 1e-5
N_CORES = 8

TOK_TILES = N_TOK // 128  # 32
D_TILES = D_CTX // 128  # 8
E_TILES = INNER // 128  # 6
QUARTERS = 4  # token quarters for xnT staging
QT_TOK = N_TOK // QUARTERS  # 1024 tokens per quarter


def emit_kernel(ctx, tc, out_d, x_d, wp_d, qt_d, wout_d, rrep_d):
    nc = tc.nc
    rc_dram = nc.dram_tensor("rc_scratch", [N_HEAD, NQ], F32).ap()

    p_wp = ctx.enter_context(tc.tile_pool(name="wp", bufs=1))
    p_qt = ctx.enter_context(tc.tile_pool(name="qt", bufs=1))
    p_r = ctx.enter_context(tc.tile_pool(name="rr", bufs=1))
    p_x = ctx.enter_context(tc.tile_pool(name="x", bufs=2))
    p_xn = ctx.enter_context(tc.tile_pool(name="xn", bufs=2))
    p_big = ctx.enter_context(tc.tile_pool(name="big", bufs=2))
    p_kt = ctx.enter_context(tc.tile_pool(name="kt", bufs=E_TILES))
    p_v = ctx.enter_context(tc.tile_pool(name="v", bufs=TOK_TILES))
    p_stat = ctx.enter_context(tc.tile_pool(name="stat", bufs=4))
    p_ot = ctx.enter_context(tc.tile_pool(name="ot", bufs=N_HEAD))
    p_fin = ctx.enter_context(tc.tile_pool(name="fin", bufs=2))
    p_rc = ctx.enter_context(tc.tile_pool(name="rc", bufs=2))
    ps_kv = ctx.enter_context(tc.tile_pool(name="pskv", bufs=2, space="PSUM"))
    ps_sim = ctx.enter_context(tc.tile_pool(name="pssim", bufs=2, space="PSUM"))
    ps_av = ctx.enter_context(tc.tile_pool(name="psav", bufs=2, space="PSUM"))

    # --- constant loads -------------------------------------------------
    wp = p_wp.tile([128, D_TILES, 2 * INNER], BF16, tag="wp")
    nc.gpsimd.dma_start(out=wp[:], in_=wp_d.rearrange("(t p) n -> p t n", p=128))
    qt = p_qt.tile([128, E_TILES, NQ], BF16)
    nc.gpsimd.dma_start(out=qt[:], in_=qt_d.rearrange("(t p) n -> p t n", p=128))
    eps_t = p_qt.tile([128, 1], F32, tag="eps")
    nc.vector.memset(eps_t[:], EPS)

    kt_tiles = []
    for e in range(E_TILES):
        kt_tiles.append(p_kt.tile([128, N_TOK], BF16, tag="kt", name=f"kt{e}"))
    v_tiles = []
    for j in range(TOK_TILES):
        v_tiles.append(p_v.tile([128, N_HEAD, DH + 1], BF16, tag="v", name=f"v{j}"))

    # --- phase 1+2: LN(x) -> xnT quarters -> kT / v ---------------------
    xnt = None
    for i in range(TOK_TILES):
        q = i // (QT_TOK // 128)
        w = i % (QT_TOK // 128)

        xt = p_x.tile([128, D_CTX], F32, tag="x")
        nc.gpsimd.dma_start(out=xt[:], in_=x_d[i * 128 : (i + 1) * 128, :])

        st = p_stat.tile([128, 2, 6], F32, tag="st")
        nc.vector.bn_stats(out=st[:, 0, :], in_=xt[:, 0:512])
        nc.vector.bn_stats(out=st[:, 1, :], in_=xt[:, 512:1024])
        mv = p_stat.tile([128, 2], F32, tag="mv")
        nc.vector.bn_aggr(out=mv[:], in_=st[:])
        rstd = p_stat.tile([128, 1], F32, tag="rstd")
        nc.scalar.activation(
            out=rstd[:],
            in_=mv[:, 1:2],
            func=mybir.ActivationFunctionType.Sqrt,
            bias=eps_t[:],
            scale=1.0,
        )
        nc.vector.reciprocal(out=rstd[:], in_=rstd[:])
        negmr = p_stat.tile([128, 1], F32, tag="negmr")
        nc.vector.scalar_tensor_tensor(
            out=negmr[:],
            in0=mv[:, 0:1],
            scalar=-1.0,
            in1=rstd[:],
            op0=AX.mult,
            op1=AX.mult,
        )

        xn = p_xn.tile([128, D_CTX], BF16, tag="xn")
        nc.vector.tensor_scalar(
            out=xn[:],
            in0=xt[:],
            scalar1=rstd[:, 0:1],
            scalar2=negmr[:, 0:1],
            op0=AX.mult,
            op1=AX.add,
        )

        if w == 0:
            xnt = p_big.tile([128, D_TILES, QT_TOK], BF16, tag="big")
        for d in range(D_TILES):
            nc.sync.dma_start(
                out=xnt[:, d, w * 128 : (w + 1) * 128],
                in_=xn[:, d * 128 : (d + 1) * 128],
                transpose=True,
            )

        if w == (QT_TOK // 128) - 1:
            # kT for this token quarter
            for e in range(E_TILES):
                for n2 in range(QT_TOK // 512):
                    ps = ps_kv.tile([128, 512], F32, tag="ps")
                    for d in range(D_TILES):
                        nc.tensor.matmul(
                            out=ps[:],
                            lhsT=wp[:, d, e * 128 : (e + 1) * 128],
                            rhs=xnt[:, d, n2 * 512 : (n2 + 1) * 512],
                            start=(d == 0),
                            stop=(d == D_TILES - 1),
                        )
                    c0 = q * QT_TOK + n2 * 512
                    nc.vector.tensor_copy(out=kt_tiles[e][:, c0 : c0 + 512], in_=ps[:])
            # v for this token quarter
            for j8 in range(QT_TOK // 128):
                j = q * (QT_TOK // 128) + j8
                vt = v_tiles[j]
                nc.vector.memset(vt[:, :, DH : DH + 1], 1.0)
                for h6 in range(2):
                    ps = ps_kv.tile([128, 384], F32, tag="ps")
                    for d in range(D_TILES):
                        nc.tensor.matmul(
                            out=ps[:],
                            lhsT=xnt[:, d, j8 * 128 : (j8 + 1) * 128],
                            rhs=wp[:, d, INNER + h6 * 384 : INNER + (h6 + 1) * 384],
                            start=(d == 0),
                            stop=(d == D_TILES - 1),
                        )
                    nc.vector.tensor_copy(
                        out=vt[:, h6 * 6 : (h6 + 1) * 6, 0:DH],
                        in_=ps.rearrange("p (h dh) -> p h dh", dh=DH),
                    )

    # --- constants needed from the attention phase on -------------------
    # wout reuses the wp slot (kv projection no longer needs the weights).
    wout = p_wp.tile([DH, N_HEAD, D_MODEL], BF16, tag="wp")
    nc.gpsimd.dma_start(out=wout[:], in_=wout_d[:])
    rrep = p_r.tile([128, D_MODEL], F32)
    nc.gpsimd.dma_start(out=rrep[:], in_=rrep_d[:])

    # --- phase 3: attention per head pair -------------------------------
    ot_tiles = []
    for p in range(E_TILES):
        attn_ab = []
        for base in (0, 64):
            attn = p_big.tile([128, TOK_TILES, NQ], BF16, tag="big", name=f"attn{p}_{base}")
            attn_ab.append(attn)
            for jg in range(TOK_TILES // 4):
                ps = ps_sim.tile([128, 4, NQ], F32, tag="ps")
                for jj in range(4):
                    j = jg * 4 + jj
                    nc.tensor.matmul(
                        out=ps[:, jj, :],
                        lhsT=kt_tiles[p][base : base + 64, j * 128 : (j + 1) * 128],
                        rhs=qt[base : base + 64, p, :],
                        start=True,
                        stop=True,
                    )
                nc.scalar.activation(
                    out=attn[:, jg * 4 : (jg + 1) * 4, :],
                    in_=ps[:],
                    func=mybir.ActivationFunctionType.Exp,
                )
        for hh in range(2):
            h = 2 * p + hh
            attn = attn_ab[hh]
            psav = ps_av.tile([DH + 1, NQ], F32, tag="ps")
            for j in range(TOK_TILES):
                nc.tensor.matmul(
                    out=psav[:],
                    lhsT=v_tiles[j][:, h, :],
                    rhs=attn[:, j, :],
                    start=(j == 0),
                    stop=(j == TOK_TILES - 1),
                )
            rc_sb = p_rc.tile([128, NQ], F32, tag="rcsb")
            nc.vector.reciprocal(out=rc_sb[DH : DH + 1, :], in_=psav[DH : DH + 1, :])
            # partition-broadcast via a DRAM bounce (SBUF->SBUF 0-stride
            # partition sources are not supported by the DMA lowering)
            nc.sync.dma_start(out=rc_dram[h : h + 1, :], in_=rc_sb[DH : DH + 1, :])
            rcrep = p_rc.tile([DH, NQ], F32, tag="rcrep")
            nc.gpsimd.dma_start(
                out=rcrep[:], in_=rc_dram[h : h + 1, :].to_broadcast([DH, NQ])
            )
            ot = p_ot.tile([DH, NQ], BF16, tag="ot", name=f"ot{h}")
            nc.vector.tensor_tensor(
                out=ot[:], in0=psav[0:DH, :], in1=rcrep[:], op=AX.mult
            )
            ot_tiles.append(ot)

    # --- phase 4: output projection -------------------------------------
    for q2 in range(NQ // 128):
        fin = p_fin.tile([128, D_MODEL], F32, tag="fin")
        for n2 in range(2):
            psf = ps_kv.tile([128, 384], F32, tag="ps")
            for h in range(N_HEAD):
                nc.tensor.matmul(
                    out=psf[:],
                    lhsT=ot_tiles[h][:, q2 * 128 : (q2 + 1) * 128],
                    rhs=wout[:, h, n2 * 384 : (n2 + 1) * 384],
                    start=(h == 0),
                    stop=(h == N_HEAD - 1),
                )
            nc.vector.tensor_tensor(
                out=fin[:, n2 * 384 : (n2 + 1) * 384],
                in0=psf[:],
                in1=rrep[:, n2 * 384 : (n2 + 1) * 384],
                op=AX.add,
            )
        nc.sync.dma_start(out=out_d[q2 * 128 : (q2 + 1) * 128, :], in_=fin[:])


def build_nc():
    nc = bacc.Bacc(
        "TRN2", target_bir_lowering=False, debug=False, num_devices=N_CORES
    )
    x_d = nc.dram_tensor("x", [N_TOK, D_CTX], F32, kind="ExternalInput").ap()
    wp_d = nc.dram_tensor("wp", [D_CTX, 2 * INNER], BF16, kind="ExternalInput").ap()
    qt_d = nc.dram_tensor("qt", [INNER, NQ], BF16, kind="ExternalInput").ap()
    wout_d = nc.dram_tensor(
        "wout", [DH, N_HEAD, D_MODEL], BF16, kind="ExternalInput"
    ).ap()
    rrep_d = nc.dram_tensor("rrep", [128, D_MODEL], F32, kind="ExternalInput").ap()
    out_d = nc.dram_tensor("out", [NQ, D_MODEL], F32, kind="ExternalOutput").ap()
    from contextlib import ExitStack

    with tile.TileContext(nc) as tc:
        with ExitStack() as ctx:
            emit_kernel(ctx, tc, out_d, x_d, wp_d, qt_d, wout_d, rrep_d)
    nc.compile()
    return nc


def host_prep(query, ln_q_w, ln_q_b, ln_k_w, ln_k_b, Wq, Wkv, Wout):
    """Batch-independent fp32 preprocessing. Returns per-core input dict
    (minus x)."""
    query = np.asarray(query, np.float32)
    mu = query.mean(-1, keepdims=True)
    var = ((query - mu) ** 2).mean(-1, keepdims=True)
    qn = (query - mu) / np.sqrt(var + EPS) * ln_q_w + ln_q_b
    qmat = (qn @ np.asarray(Wq, np.float32)) * (DH**-0.5)  # [NQ, INNER]
    qT = np.ascontiguousarray(qmat.T).astype(ml_dtypes.bfloat16)

    Wkv = np.asarray(Wkv, np.float32)
    Wp = (np.asarray(ln_k_w, np.float32)[:, None] * Wkv).astype(ml_dtypes.bfloat16)
    c = np.asarray(ln_k_b, np.float32) @ Wkv  # [2*INNER]
    c_v = c[INNER:]
    Wout = np.asarray(Wout, np.float32)
    r = c_v @ Wout  # [D_MODEL]
    rrep = np.ascontiguousarray(np.broadcast_to(r, (128, D_MODEL))).astype(np.float32)
    wout_arr = np.ascontiguousarray(
        Wout.reshape(N_HEAD, DH, D_MODEL).transpose(1, 0, 2)
    ).astype(ml_dtypes.bfloat16)
    return {"wp": Wp, "qt": qT, "wout": wout_arr, "rrep": rrep}


_NC_CACHE = {}


def get_nc():
    if "nc" not in _NC_CACHE:
        _NC_CACHE["nc"] = build_nc()
    return _NC_CACHE["nc"]


def kernel(x, query, ln_q_w, ln_q_b, ln_k_w, ln_k_b, Wq, Wkv, Wout):
    x = np.asarray(x, np.float32)
    shared = host_prep(query, ln_q_w, ln_q_b, ln_k_w, ln_k_b, Wq, Wkv, Wout)
    in_maps = [
        {"x": np.ascontiguousarray(x[b]), **shared} for b in range(B)
    ]
    nc = get_nc()
    res = run_bass_kernel_spmd(nc, in_maps, list(range(N_CORES)))
    return np.stack([res.results[b]["out"] for b in range(B)], axis=0)


# revision 17
# speedup vs baseline: 341.3379x; 341.3379x over previous
"""AttentionalPooler Trainium2 kernel.

Full inputs -> full output; batch (8) is data-parallel across the 8
NeuronCores. Per core: LayerNorm(x_b), kv = LN(x_b) @ Wkv, 12-head
cross-attention from 256 pre-computed queries, output projection.

Host-side preprocessing (exact fp32 algebra, batch-independent):
  - q path (LN(query) @ Wq * dh^-0.5, transposed) is computed on host.
  - ln_k_w is folded into the kv weights (Wp = diag(ln_k_w) @ Wkv).
  - ln_k_b folds into c = ln_k_b @ Wkv. The k-part of c shifts every
    logit of a (head, query) row by the same constant, which softmax
    cancels exactly, so it is dropped. The v-part adds c_v to every
    attention output row (attention weights sum to 1), so it commutes
    past Wout: the kernel adds r = c_v @ Wout to the final output.

Device schedule (single pass, engines in-order per their streams):
  - Phase A (per 1024-token quarter): cast-load x to bf16, LayerNorm on
    DVE, bounce normalized tiles through DRAM, 8 large xbar DMA-
    transposes into xnT, then the K-projection matmuls (kT, e-major).
  - Phase B (per quarter): re-transpose xnT from the DRAM bounce, the
    V-projection matmuls, then for all 6 head pairs the sim matmuls
    (simT[tok, query], K=64 row-pair packed) -> exp on ACT -> the
    attn@v chunk matmuls accumulating into per-head SBUF accumulators.
    Emitting sim for all pairs before attn@v keeps PE from stalling on
    ACT, and interleaving per quarter lets exp hide under the V matmuls.
  - Softmax denominators come from a ones-column appended to v; max-
    subtraction is skipped (logits provably small for LN'd inputs).
  - Phase C: per-head normalize via reciprocal + DRAM-bounce partition
    broadcast. Phase D: output projection (+ the c_v@Wout constant).
"""

import sys

sys.path.insert(0, "/opt/trn_rl_repo")

import numpy as np
import ml_dtypes

import concourse.bass as bass
import concourse.mybir as mybir
import concourse.tile as tile
from concourse import bacc
from concourse.bass_utils import run_bass_kernel_spmd

F32 = mybir.dt.float32
BF16 = mybir.dt.bfloat16
AX = mybir.AluOpType

B = 8
N_TOK = 4096
D_CTX = 1024
D_MODEL = 768
N_HEAD = 12
DH = 64
NQ = 256
INNER = 768
EPS = 1e-5
N_CORES = 8

TOK_TILES = N_TOK // 128  # 32
D_TILES = D_CTX // 128  # 8
E_TILES = INNER // 128  # 6
QUARTERS = 4
QT_TOK = N_TOK // QUARTERS  # 1024
QJ = QT_TOK // 128  # 8 token tiles per quarter


def emit_kernel(ctx, tc, out_d, x_d, wp_d, qt_d, wout_d, rrep_d, rep=0):
    nc = tc.nc
    rc_dram = nc.dram_tensor(f"rc_scratch{rep}", [N_HEAD, NQ], F32).ap()
    xn_dram = nc.dram_tensor(f"xn_scratch{rep}", [N_TOK, D_CTX], BF16).ap()

    p_wp = ctx.enter_context(tc.tile_pool(name="wp", bufs=1))
    p_qt = ctx.enter_context(tc.tile_pool(name="qt", bufs=1))
    p_r = ctx.enter_context(tc.tile_pool(name="rr", bufs=1))
    p_x = ctx.enter_context(tc.tile_pool(name="x", bufs=3))
    p_xn = ctx.enter_context(tc.tile_pool(name="xn", bufs=2))
    p_big = ctx.enter_context(tc.tile_pool(name="big", bufs=2))
    p_attn = ctx.enter_context(tc.tile_pool(name="attn", bufs=3))
    p_kt = ctx.enter_context(tc.tile_pool(name="kt", bufs=E_TILES))
    p_v = ctx.enter_context(tc.tile_pool(name="v", bufs=TOK_TILES))
    p_acc = ctx.enter_context(tc.tile_pool(name="acc", bufs=N_HEAD))
    p_stat = ctx.enter_context(tc.tile_pool(name="stat", bufs=4))
    p_ot = ctx.enter_context(tc.tile_pool(name="ot", bufs=N_HEAD))
    p_fin = ctx.enter_context(tc.tile_pool(name="fin", bufs=1))
    p_rc = ctx.enter_context(tc.tile_pool(name="rc", bufs=2))
    ps_kv = ctx.enter_context(tc.tile_pool(name="pskv", bufs=2, space="PSUM"))
    ps_sim = ctx.enter_context(tc.tile_pool(name="pssim", bufs=2, space="PSUM"))
    ps_av = ctx.enter_context(tc.tile_pool(name="psav", bufs=2, space="PSUM"))

    # --- LN(x) preprocessing for one x-tile ------------------------------
    def prep_tile(i):
        xt = p_x.tile([128, D_CTX], BF16, tag="x", name=f"x{i}")
        nc.gpsimd.dma_start(out=xt[:], in_=x_d[i * 128 : (i + 1) * 128, :])

        st = p_stat.tile([128, 2, 6], F32, tag="st", name=f"st{i}")
        nc.vector.bn_stats(out=st[:, 0, :], in_=xt[:, 0:512])
        nc.vector.bn_stats(out=st[:, 1, :], in_=xt[:, 512:1024])
        mv = p_stat.tile([128, 2], F32, tag="mv", name=f"mv{i}")
        nc.vector.bn_aggr(out=mv[:], in_=st[:])
        rstd = p_stat.tile([128, 1], F32, tag="rstd", name=f"rstd{i}")
        nc.scalar.activation(
            out=rstd[:],
            in_=mv[:, 1:2],
            func=mybir.ActivationFunctionType.Sqrt,
            bias=eps_t[:],
            scale=1.0,
        )
        nc.vector.reciprocal(out=rstd[:], in_=rstd[:])
        negmr = p_stat.tile([128, 1], F32, tag="negmr", name=f"negmr{i}")
        nc.vector.scalar_tensor_tensor(
            out=negmr[:],
            in0=mv[:, 0:1],
            scalar=-1.0,
            in1=rstd[:],
            op0=AX.mult,
            op1=AX.mult,
        )
        xn = p_xn.tile([128, D_CTX], BF16, tag="xn", name=f"xn{i}")
        nc.vector.tensor_scalar(
            out=xn[:],
            in0=xt[:],
            scalar1=rstd[:, 0:1],
            scalar2=negmr[:, 0:1],
            op0=AX.mult,
            op1=AX.add,
        )
        nc.sync.dma_start(out=xn_dram[i * 128 : (i + 1) * 128, :], in_=xn[:])

    def load_xnt(q, phase):
        xnt = p_big.tile([128, D_TILES, QT_TOK], BF16, tag="big",
                         name=f"xnt{phase}{q}")
        for d in range(D_TILES):
            nc.sync.dma_start(
                out=xnt[:, d, :],
                in_=xn_dram[q * QT_TOK : (q + 1) * QT_TOK, d * 128 : (d + 1) * 128],
                transpose=True,
            )
        return xnt

    # --- get the x pipeline going before the big weight loads ------------
    eps_t = None  # placed below; prep_tile uses it via closure
    eps_t = tc.tile_pool(name="eps", bufs=1)
    p_eps = ctx.enter_context(eps_t)
    eps_t = p_eps.tile([128, 1], F32, tag="eps")
    nc.vector.memset(eps_t[:], EPS)
    for i in range(6):
        prep_tile(i)

    # --- constant loads (after the quarter-0 x pipeline is in flight) ----
    wp = p_wp.tile([128, D_TILES, 2 * INNER], BF16, tag="wp")
    nc.gpsimd.dma_start(out=wp[:], in_=wp_d.rearrange("(t p) n -> p t n", p=128))
    qt = p_qt.tile([128, E_TILES, NQ], BF16)
    nc.gpsimd.dma_start(out=qt[:], in_=qt_d.rearrange("(t p) n -> p t n", p=128))

    kt_tiles = []
    for e in range(E_TILES):
        kt_tiles.append(p_kt.tile([128, N_TOK], BF16, tag="kt", name=f"kt{e}"))
    v_tiles = []
    for j in range(TOK_TILES):
        v_tiles.append(p_v.tile([128, N_HEAD, DH + 1], BF16, tag="v", name=f"v{j}"))

    # --- phase A: LN(x) -> xnT -> kT -------------------------------------
    for q in range(QUARTERS):
        for jj in range(QJ):
            i = q * QJ + jj + 6
            if i < TOK_TILES:
                prep_tile(i)
        xnt = load_xnt(q, "a")
        for e in range(E_TILES):
            for n2 in range(QT_TOK // 512):
                ps = ps_kv.tile([128, 512], F32, tag="ps", name=f"pkt{q}_{e}_{n2}")
                for d in range(D_TILES):
                    nc.tensor.matmul(
                        out=ps[:],
                        lhsT=wp[:, d, e * 128 : (e + 1) * 128],
                        rhs=xnt[:, d, n2 * 512 : (n2 + 1) * 512],
                        start=(d == 0),
                        stop=(d == D_TILES - 1),
                    )
                c0 = q * QT_TOK + n2 * 512
                nc.vector.tensor_copy(out=kt_tiles[e][:, c0 : c0 + 512], in_=ps[:])

    # --- attention constants ---------------------------------------------
    av_acc = []
    for h in range(N_HEAD):
        av_acc.append(p_acc.tile([DH + 1, NQ], F32, tag="acc", name=f"acc{h}"))

    # --- phase B: per quarter: v, then sim+exp, then attn@v chunks -------
    for q in range(QUARTERS):
        xnt = load_xnt(q, "b")
        for jj in range(QJ):
            j = q * QJ + jj
            vt = v_tiles[j]
            nc.vector.memset(vt[:, :, DH : DH + 1], 1.0)
            for h6 in range(2):
                ps = ps_kv.tile([128, 384], F32, tag="ps", name=f"pv{j}_{h6}")
                for d in range(D_TILES):
                    nc.tensor.matmul(
                        out=ps[:],
                        lhsT=xnt[:, d, jj * 128 : (jj + 1) * 128],
                        rhs=wp[:, d, INNER + h6 * 384 : INNER + (h6 + 1) * 384],
                        start=(d == 0),
                        stop=(d == D_TILES - 1),
                    )
                nc.vector.tensor_copy(
                    out=vt[:, h6 * 6 : (h6 + 1) * 6, 0:DH],
                    in_=ps.rearrange("p (h dh) -> p h dh", dh=DH),
                )

        # sim + exp for all pairs over this quarter's 8 token tiles
        attn_tiles = {}
        for p in range(E_TILES):
            for hh, base in ((0, 0), (1, 64)):
                attn = p_attn.tile([128, QJ, NQ], BF16, tag="attn",
                                   name=f"at{q}_{p}_{hh}")
                attn_tiles[(p, hh)] = attn
                for g in range(QJ // 4):
                    ps = ps_sim.tile([128, 4, NQ], F32, tag="ps",
                                     name=f"psim{q}_{p}_{hh}_{g}")
                    for jj in range(4):
                        j = q * QJ + g * 4 + jj
                        nc.tensor.matmul(
                            out=ps[:, jj, :],
                            lhsT=kt_tiles[p][base : base + 64,
                                             j * 128 : (j + 1) * 128],
                            rhs=qt[base : base + 64, p, :],
                            start=True,
                            stop=True,
                        )
                    nc.scalar.activation(
                        out=attn[:, g * 4 : (g + 1) * 4, :],
                        in_=ps[:],
                        func=mybir.ActivationFunctionType.Exp,
                    )
        # attn@v chunks for all pairs
        for p in range(E_TILES):
            for hh in range(2):
                h = 2 * p + hh
                attn = attn_tiles[(p, hh)]
                psa = ps_av.tile([DH + 1, NQ], F32, tag="ps", name=f"pav{q}_{h}")
                for jj in range(QJ):
                    nc.tensor.matmul(
                        out=psa[:],
                        lhsT=v_tiles[q * QJ + jj][:, h, :],
                        rhs=attn[:, jj, :],
                        start=(jj == 0),
                        stop=(jj == QJ - 1),
                    )
                if q == 0:
                    nc.vector.tensor_copy(out=av_acc[h][:], in_=psa[:])
                else:
                    nc.vector.tensor_tensor(
                        out=av_acc[h][:], in0=av_acc[h][:], in1=psa[:], op=AX.add
                    )

    # wout reuses the wp slot (projections no longer need the kv weights)
    wout = p_wp.tile([DH, N_HEAD, D_MODEL], BF16, tag="wp")
    nc.gpsimd.dma_start(out=wout[:], in_=wout_d[:])
    rrep = p_r.tile([128, D_MODEL], F32)
    nc.gpsimd.dma_start(out=rrep[:], in_=rrep_d[:])

    # --- phase C: per-head softmax normalize -----------------------------
    ot_tiles = []
    for h in range(N_HEAD):
        rc_sb = p_rc.tile([128, NQ], F32, tag="rcsb", name=f"rc{h}")
        nc.vector.reciprocal(out=rc_sb[DH : DH + 1, :], in_=av_acc[h][DH : DH + 1, :])
        nc.sync.dma_start(out=rc_dram[h : h + 1, :], in_=rc_sb[DH : DH + 1, :])
        nc.gpsimd.dma_start(
            out=rc_sb[0:DH, :], in_=rc_dram[h : h + 1, :].to_broadcast([DH, NQ])
        )
        ot = p_ot.tile([DH, NQ], BF16, tag="ot", name=f"ot{h}")
        nc.vector.tensor_tensor(
            out=ot[:], in0=av_acc[h][0:DH, :], in1=rc_sb[0:DH, :], op=AX.mult
        )
        ot_tiles.append(ot)

    # --- phase D: output projection --------------------------------------
    for q2 in range(NQ // 128):
        fin = p_fin.tile([128, D_MODEL], F32, tag="fin", name=f"fin{q2}")
        for n2 in range(2):
            psf = ps_kv.tile([128, 384], F32, tag="ps", name=f"pf{q2}_{n2}")
            for h in range(N_HEAD):
                nc.tensor.matmul(
                    out=psf[:],
                    lhsT=ot_tiles[h][:, q2 * 128 : (q2 + 1) * 128],
                    rhs=wout[:, h, n2 * 384 : (n2 + 1) * 384],
                    start=(h == 0),
                    stop=(h == N_HEAD - 1),
                )
            nc.vector.tensor_tensor(
                out=fin[:, n2 * 384 : (n2 + 1) * 384],
                in0=psf[:],
                in1=rrep[:, n2 * 384 : (n2 + 1) * 384],
                op=AX.add,
            )
        nc.sync.dma_start(out=out_d[q2 * 128 : (q2 + 1) * 128, :], in_=fin[:])


def build_nc(reps=1):
    nc = bacc.Bacc(
        "TRN2", target_bir_lowering=False, debug=False, num_devices=N_CORES
    )
    x_d = nc.dram_tensor("x", [N_TOK, D_CTX], F32, kind="ExternalInput").ap()
    wp_d = nc.dram_tensor("wp", [D_CTX, 2 * INNER], BF16, kind="ExternalInput").ap()
    qt_d = nc.dram_tensor("qt", [INNER, NQ], BF16, kind="ExternalInput").ap()
    wout_d = nc.dram_tensor(
        "wout", [DH, N_HEAD, D_MODEL], BF16, kind="ExternalInput"
    ).ap()
    rrep_d = nc.dram_tensor("rrep", [128, D_MODEL], F32, kind="ExternalInput").ap()
    out_d = nc.dram_tensor("out", [NQ, D_MODEL], F32, kind="ExternalOutput").ap()
    from contextlib import ExitStack

    with tile.TileContext(nc) as tc:
        for rep in range(reps):
            with ExitStack() as ctx:
                emit_kernel(ctx, tc, out_d, x_d, wp_d, qt_d, wout_d, rrep_d, rep=rep)
    nc.compile()
    return nc


def host_prep(query, ln_q_w, ln_q_b, ln_k_w, ln_k_b, Wq, Wkv, Wout):
    """Batch-independent fp32 preprocessing. Returns per-core input dict
    (minus x)."""
    query = np.asarray(query, np.float32)
    mu = query.mean(-1, keepdims=True)
    var = ((query - mu) ** 2).mean(-1, keepdims=True)
    qn = (query - mu) / np.sqrt(var + EPS) * ln_q_w + ln_q_b
    qmat = (qn @ np.asarray(Wq, np.float32)) * (DH**-0.5)  # [NQ, INNER]
    qT = np.ascontiguousarray(qmat.T).astype(ml_dtypes.bfloat16)

    Wkv = np.asarray(Wkv, np.float32)
    Wp = (np.asarray(ln_k_w, np.float32)[:, None] * Wkv).astype(ml_dtypes.bfloat16)
    c = np.asarray(ln_k_b, np.float32) @ Wkv  # [2*INNER]
    c_v = c[INNER:]
    Wout = np.asarray(Wout, np.float32)
    r = c_v @ Wout  # [D_MODEL]
    rrep = np.ascontiguousarray(np.broadcast_to(r, (128, D_MODEL))).astype(np.float32)
    wout_arr = np.ascontiguousarray(
        Wout.reshape(N_HEAD, DH, D_MODEL).transpose(1, 0, 2)
    ).astype(ml_dtypes.bfloat16)
    return {"wp": Wp, "qt": qT, "wout": wout_arr, "rrep": rrep}


_NC_CACHE = {}


def get_nc():
    if "nc" not in _NC_CACHE:
        _NC_CACHE["nc"] = build_nc()
    return _NC_CACHE["nc"]


def kernel(x, query, ln_q_w, ln_q_b, ln_k_w, ln_k_b, Wq, Wkv, Wout):
    x = np.asarray(x, np.float32)
    shared = host_prep(query, ln_q_w, ln_q_b, ln_k_w, ln_k_b, Wq, Wkv, Wout)
    in_maps = [
        {"x": np.ascontiguousarray(x[b]), **shared} for b in range(B)
    ]
    nc = get_nc()
    res = run_bass_kernel_spmd(nc, in_maps, list(range(N_CORES)))
    return np.stack([res.results[b]["out"] for b in range(B)], axis=0)
